# revision 1
# baseline (speedup 1.0000x reference)
"""DistMult scoring kernel for Trainium2 (8 NeuronCores, SPMD data-parallel).

Problem: score = sigmoid( (ent_emb[h] * diag(rel_emb[r])) @ ent_emb[t].T )
  batch_h/t/r: (2048,) int indices; ent_emb: (400000, 256) f32;
  rel_emb: (500, 256, 256) f32 but diagonal -> only its (500, 256) diagonal matters.

Sharding: data-parallel over the heads axis. Each of the 8 cores gathers its
256 head rows + rel diagonals and all 2048 tail rows from DRAM (indirect DMA),
computes a (256, 2048) block of scores, host concatenates row blocks.

Kernel structure (per core):
  - 20 indirect-DMA gathers (one [128,1]-index call each; multi-index calls
    are broken in the SWDGE ucode). Order: tails[0:4], heads, rels,
    tails[4:16] so the first score chunk and the hr path both start early.
  - PE-transposes (via identity matmul) put the contraction dim on partitions
    for hr and tails; fp32 matmuls accumulate over 2 K-tiles into PSUM.
  - ScalarE applies sigmoid straight out of PSUM; HWDGE writes each chunk.
  - A dummy-matmul warmup chain keeps the PE HAM clock-gate at K=8/8.
"""

import sys

if "/opt/trn_rl_repo" not in sys.path:
    sys.path.insert(0, "/opt/trn_rl_repo")

import numpy as np

import concourse.bass as bass
import concourse.tile as tile
from concourse import bacc, mybir

B = 2048          # batch size
E = 256           # embedding dim
N_ENT = 400000
N_REL = 500
CORES = 8
M = B // CORES    # head rows per core = 256
P = 128

F32 = mybir.dt.float32
I32 = mybir.dt.int32

NT = B // P       # 16 tail index columns
NM = M // P       # 2 head tiles
NK = E // P       # 2 contraction tiles


def build_nc():
    nc = bacc.Bacc("TRN2", target_bir_lowering=False, debug=False, num_devices=CORES)

    # idx layout: cols [0:16] tails, [16:18] heads, [18:20] rels
    idx = nc.dram_tensor("idx", [P, NT + 2 * NM], I32, kind="ExternalInput").ap()
    identity = nc.dram_tensor("identity", [P, P], F32, kind="ExternalInput").ap()
    ent = nc.dram_tensor("ent_emb", [N_ENT, E], F32, kind="ExternalInput").ap()
    rel = nc.dram_tensor("rel_diag", [N_REL, E], F32, kind="ExternalInput").ap()
    score = nc.dram_tensor("score", [M, B], F32, kind="ExternalOutput").ap()

    with tile.TileContext(nc) as tc:
        with (
            tc.tile_pool(name="const", bufs=1) as const_pool,
            tc.tile_pool(name="idxp", bufs=1) as idx_pool,
            tc.tile_pool(name="gather", bufs=1) as gather_pool,
            tc.tile_pool(name="tailsT", bufs=1) as tailsT_pool,
            tc.tile_pool(name="outp", bufs=8) as out_pool,
            tc.tile_pool(name="pst", bufs=3, space="PSUM") as psum_t,
            tc.tile_pool(name="psmm", bufs=4, space="PSUM") as psum_mm,
            tc.tile_pool(name="pswm", bufs=1, space="PSUM") as psum_wm,
        ):
            idx_sb = idx_pool.tile([P, NT + 2 * NM], I32)
            # col 0 first so the first gather's dependency lands ASAP
            nc.sync.dma_start(idx_sb[:, 0:1], idx[:, 0:1])
            nc.sync.dma_start(idx_sb[:, 1:], idx[:, 1:])
            ident = const_pool.tile([P, P], F32)
            nc.sync.dma_start(ident[:], identity[:])

            # --- gather order: tails chunk 0, heads+rels, remaining tails ---
            tails = [None] * NT

            def gather_tail(j):
                t_tile = gather_pool.tile([P, E], F32, tag=f"tails{j}", name=f"t{j}")
                nc.gpsimd.indirect_dma_start(
                    out=t_tile[:],
                    out_offset=None,
                    in_=ent[:],
                    in_offset=bass.IndirectOffsetOnAxis(
                        ap=idx_sb[:, j : j + 1], axis=0
                    ),
                )
                tails[j] = t_tile

            for j in range(4):
                gather_tail(j)

            h_tiles, r_tiles = [], []
            for i in range(NM):
                h_tile = gather_pool.tile([P, E], F32, tag=f"heads{i}")
                nc.gpsimd.indirect_dma_start(
                    out=h_tile[:],
                    out_offset=None,
                    in_=ent[:],
                    in_offset=bass.IndirectOffsetOnAxis(
                        ap=idx_sb[:, NT + i : NT + i + 1], axis=0
                    ),
                )
                h_tiles.append(h_tile)
                r_tile = gather_pool.tile([P, E], F32, tag=f"rels{i}")
                nc.gpsimd.indirect_dma_start(
                    out=r_tile[:],
                    out_offset=None,
                    in_=rel[:],
                    in_offset=bass.IndirectOffsetOnAxis(
                        ap=idx_sb[:, NT + NM + i : NT + NM + i + 1], axis=0
                    ),
                )
                r_tiles.append(r_tile)

            for j in range(4, NT):
                gather_tail(j)

            # --- PE warmup: dummy accumulating matmuls on the identity keep the
            # HAM clock-gate busy (K=8/8) until real transposes arrive ---
            wm = psum_wm.tile([P, P], F32)
            for _ in range(64):
                nc.tensor.matmul(wm[:], lhsT=ident[:], rhs=ident[:], start=True, stop=True)

            # --- hr = heads * rel; transpose -> hrT[k] = [128(k), 256(m)] ---
            hr_tiles = []
            for i in range(NM):
                hr_tile = gather_pool.tile([P, E], F32, tag=f"hr{i}")
                nc.vector.tensor_mul(hr_tile[:], h_tiles[i][:], r_tiles[i][:])
                hr_tiles.append(hr_tile)

            hrT = []
            for k in range(NK):
                hrT_k = gather_pool.tile([P, M], F32, tag=f"hrT{k}")
                pst = psum_t.tile([P, M], F32, tag="pst", name=f"pst_hr{k}")
                for i in range(NM):
                    nc.tensor.transpose(
                        pst[:, i * P : (i + 1) * P],
                        hr_tiles[i][:, k * P : (k + 1) * P],
                        ident[:],
                    )
                nc.vector.tensor_copy(hrT_k[:], pst[:])
                hrT.append(hrT_k)

            # --- per n-chunk: transpose tails, matmul, sigmoid, out ---
            tailsT = []
            for k in range(NK):
                tailsT_k = tailsT_pool.tile([P, B], F32, tag=f"tailsT{k}")
                tailsT.append(tailsT_k)

            # chunk widths in tail-gather columns; narrow trailing chunks keep
            # the post-last-gather critical chain short
            widths = [4, 4, 4, 2, 1, 1]
            j0 = 0
            for w in widths:
                ncols = w * P
                for k in range(NK):
                    pst = psum_t.tile([P, ncols], F32, tag="pst", name=f"pst_{j0}_{k}")
                    for jj in range(w):
                        j = j0 + jj
                        nc.tensor.transpose(
                            pst[:, jj * P : (jj + 1) * P],
                            tails[j][:, k * P : (k + 1) * P],
                            ident[:],
                        )
                    nc.vector.tensor_copy(
                        tailsT[k][:, j0 * P : j0 * P + ncols], pst[:]
                    )

                for i in range(NM):
                    psmm = psum_mm.tile(
                        [P, ncols], F32, tag="psmm", name=f"psmm_{j0}_{i}"
                    )
                    for k in range(NK):
                        nc.tensor.matmul(
                            psmm[:],
                            lhsT=hrT[k][:, i * P : (i + 1) * P],
                            rhs=tailsT[k][:, j0 * P : j0 * P + ncols],
                            start=(k == 0),
                            stop=(k == NK - 1),
                        )
                    o_tile = out_pool.tile(
                        [P, ncols], F32, tag="out", name=f"out_{j0}_{i}"
                    )
                    nc.scalar.activation(
                        o_tile[:], psmm[:], mybir.ActivationFunctionType.Sigmoid
                    )
                    # alternate HWDGE engines so back-to-back output issues
                    # at the kernel tail don't serialize on one engine
                    out_eng = nc.sync if i == 0 else nc.scalar
                    out_eng.dma_start(
                        score[i * P : (i + 1) * P, j0 * P : j0 * P + ncols], o_tile[:]
                    )
                j0 += w

    nc.compile()
    return nc


_NC = None


def _get_nc():
    global _NC
    if _NC is None:
        _NC = build_nc()
    return _NC


def make_in_maps(batch_h, batch_t, batch_r, ent_emb, rel_emb):
    h = np.ascontiguousarray(np.asarray(batch_h), dtype=np.int32)
    t = np.ascontiguousarray(np.asarray(batch_t), dtype=np.int32)
    r = np.ascontiguousarray(np.asarray(batch_r), dtype=np.int32)
    ent = np.ascontiguousarray(np.asarray(ent_emb), dtype=np.float32)
    # rel_emb is diag_embed(rel_vec): only the diagonal carries information.
    rel_np = np.asarray(rel_emb)
    rel_diag = np.ascontiguousarray(
        rel_np[:, np.arange(E), np.arange(E)].astype(np.float32)
    )
    identity = np.eye(P, dtype=np.float32)

    # partition-major wrap: idx_tile[p, g] = idx[g*128 + p]
    t_wrapped = t.reshape(NT, P).T  # (128, 16)
    in_maps = []
    for c in range(CORES):
        sl = slice(c * M, (c + 1) * M)
        idx_all = np.concatenate(
            [t_wrapped, h[sl].reshape(NM, P).T, r[sl].reshape(NM, P).T], axis=1
        )
        in_maps.append(
            {
                "idx": np.ascontiguousarray(idx_all),
                "identity": identity,
                "ent_emb": ent,
                "rel_diag": rel_diag,
            }
        )
    return in_maps


def run(batch_h, batch_t, batch_r, ent_emb, rel_emb, trace=False, tmpdir=None):
    """Run the SPMD kernel; returns (score, BassKernelResults)."""
    from concourse.bass_utils import run_bass_kernel_spmd

    nc = _get_nc()
    in_maps = make_in_maps(batch_h, batch_t, batch_r, ent_emb, rel_emb)
    kwargs = {}
    if trace:
        kwargs = {"trace": True, "tmpdir": tmpdir}
    res = run_bass_kernel_spmd(nc, in_maps, core_ids=list(range(CORES)), **kwargs)
    score = np.concatenate([res.results[c]["score"] for c in range(CORES)], axis=0)
    return score, res


def kernel(batch_h, batch_t, batch_r, ent_emb, rel_emb):
    score, _ = run(batch_h, batch_t, batch_r, ent_emb, rel_emb)
    return score



# revision 9
# speedup vs baseline: 2.1583x; 2.1583x over previous
"""DistMult scoring kernel for Trainium2 (8 NeuronCores, SPMD data-parallel).

Problem: score = sigmoid( (ent_emb[h] * diag(rel_emb[r])) @ ent_emb[t].T )
  batch_h/t/r: (2048,) int indices; ent_emb: (400000, 256) f32;
  rel_emb: (500, 256, 256) f32 but diagonal -> only its (500, 256) diagonal matters.

Sharding: data-parallel over the heads axis. Each of the 8 cores gathers its
256 head rows and all 2048 tail rows from DRAM (indirect DMA), computes a
(256, 2048) block of scores, host concatenates row blocks.

v2 datapath (all bf16 on the wire and in the PE; fp32 accumulation in PSUM):
  - ent_emb is converted to bf16 on the host; gathers move half the bytes.
  - rel selection runs on the PE as a one-hot matmul against a resident
    512x256 bf16 rel-diagonal table (saves 2 SWDGE gathers; SWDGE has ~1us
    fixed cost per instruction). The one-hot matmul also produces rel rows
    directly TRANSPOSED ([e, m]), so only the heads need PE transposes on
    the hr path.
  - PE transposes and score matmuls run in bf16: 1 cycle/row instead of
    fp32's 2 (transpose) and 4 (matmul).
  - sigmoid on ScalarE reads fp32 PSUM, writes bf16 tiles; output DMA is
    bf16 and the host widens to f32.
"""

import sys

if "/opt/trn_rl_repo" not in sys.path:
    sys.path.insert(0, "/opt/trn_rl_repo")

import numpy as np
import ml_dtypes

import concourse.bass as bass
import concourse.tile as tile
from concourse import bacc, mybir

B = 2048          # batch size
E = 256           # embedding dim
N_ENT = 400000
N_REL = 500
R_PAD = 512       # rel table padded to 4 partition tiles
CORES = 8
M = B // CORES    # head rows per core = 256
P = 128

F32 = mybir.dt.float32
BF16 = mybir.dt.bfloat16
I32 = mybir.dt.int32

NT = B // P       # 16 tail index columns
NM = M // P       # 2 head tiles
NK = E // P       # 2 contraction tiles
NR = R_PAD // P   # 4 rel-table k tiles

WARMUP = 28       # dummy PE matmuls to hold the HAM clock gate open


def build_nc():
    nc = bacc.Bacc("TRN2", target_bir_lowering=False, debug=False, num_devices=CORES)

    # idx layout: cols [0:2] heads, [2:18] tails
    idx = nc.dram_tensor("idx", [P, NM + NT], I32, kind="ExternalInput").ap()
    identity = nc.dram_tensor("identity", [P, P], BF16, kind="ExternalInput").ap()
    ent = nc.dram_tensor("ent_emb", [N_ENT, E], BF16, kind="ExternalInput").ap()
    # rel_tab[p, j*E:(j+1)*E] = rel_diag_padded[j*128 + p]
    rel_tab = nc.dram_tensor("rel_tab", [P, NR * E], BF16, kind="ExternalInput").ap()
    # onehot[p, j*M + m] = 1.0 iff batch_r[core_slice][m] == j*128 + p
    onehot = nc.dram_tensor("onehot", [P, NR * M], BF16, kind="ExternalInput").ap()
    score = nc.dram_tensor("score", [M, B], BF16, kind="ExternalOutput").ap()

    with tile.TileContext(nc) as tc:
        with (
            tc.tile_pool(name="const", bufs=1) as const_pool,
            tc.tile_pool(name="idxp", bufs=1) as idx_pool,
            tc.tile_pool(name="gather", bufs=1) as gather_pool,
            tc.tile_pool(name="tailsT", bufs=1) as tailsT_pool,
            tc.tile_pool(name="outp", bufs=8) as out_pool,
            tc.tile_pool(name="pst", bufs=2, space="PSUM") as psum_t,
            tc.tile_pool(name="psmm", bufs=3, space="PSUM") as psum_mm,
            tc.tile_pool(name="psrt", bufs=1, space="PSUM") as psum_rt,
            tc.tile_pool(name="pswm", bufs=1, space="PSUM") as psum_wm,
        ):
            idx_sb = idx_pool.tile([P, NM + NT], I32)
            nc.sync.dma_start(idx_sb[:], idx[:])
            ident = const_pool.tile([P, P], BF16)
            nc.scalar.dma_start(ident[:], identity[:])
            rel_sb = const_pool.tile([P, NR * E], BF16)
            nc.scalar.dma_start(rel_sb[:], rel_tab[:])
            oh_sb = const_pool.tile([P, NR * M], BF16)
            nc.sync.dma_start(oh_sb[:], onehot[:])

            # --- gathers: heads first (hr path is the longest), then tails ---
            h_tiles = []
            for i in range(NM):
                h_tile = gather_pool.tile([P, E], BF16, tag=f"heads{i}")
                nc.gpsimd.indirect_dma_start(
                    out=h_tile[:],
                    out_offset=None,
                    in_=ent[:],
                    in_offset=bass.IndirectOffsetOnAxis(ap=idx_sb[:, i : i + 1], axis=0),
                )
                h_tiles.append(h_tile)

            tails = [None] * NT
            for j in range(NT):
                t_tile = gather_pool.tile([P, E], BF16, tag=f"tails{j}", name=f"t{j}")
                nc.gpsimd.indirect_dma_start(
                    out=t_tile[:],
                    out_offset=None,
                    in_=ent[:],
                    in_offset=bass.IndirectOffsetOnAxis(
                        ap=idx_sb[:, NM + j : NM + j + 1], axis=0
                    ),
                )
                tails[j] = t_tile

            # --- PE warmup: dummy accumulating matmuls keep the HAM clock
            # ramping while gathers land ---
            wm = psum_wm.tile([P, P], F32)
            for _ in range(WARMUP):
                nc.tensor.matmul(wm[:], lhsT=ident[:], rhs=ident[:], start=True, stop=True)

            # --- rT[k] = rel rows selected+transposed via one-hot matmul:
            # rT[k][e, m] = sum_r rel_sb[r, k*128 + e] * onehot[r, m] ---
            rt_all = psum_rt.tile([P, NK * M], F32, tag="rt", name="rt_all")
            for k in range(NK):
                for r in range(NR):
                    nc.tensor.matmul(
                        rt_all[:, k * M : (k + 1) * M],
                        lhsT=rel_sb[:, r * E + k * P : r * E + (k + 1) * P],
                        rhs=oh_sb[:, r * M : (r + 1) * M],
                        start=(r == 0),
                        stop=(r == NR - 1),
                    )

            # --- hT[k][e, m] via PE transpose of gathered head rows ---
            ht_all = psum_rt.tile([P, NK * M], BF16, tag="ht", name="ht_all")
            for k in range(NK):
                for i in range(NM):
                    nc.tensor.transpose(
                        ht_all[:, k * M + i * P : k * M + (i + 1) * P],
                        h_tiles[i][:, k * P : (k + 1) * P],
                        ident[:],
                    )

            # --- hrT[k] = hT[k] * rT[k]; DVE reads at most one PSUM operand,
            # so bounce hT through SBUF via ScalarE first ---
            ht_sb = gather_pool.tile([P, NK * M], BF16, tag="ht_sb")
            nc.scalar.activation(
                ht_sb[:], ht_all[:], mybir.ActivationFunctionType.Copy
            )
            hrT = []
            for k in range(NK):
                hrT_k = gather_pool.tile([P, M], BF16, tag=f"hrT{k}")
                nc.vector.tensor_mul(
                    hrT_k[:],
                    ht_sb[:, k * M : (k + 1) * M],
                    rt_all[:, k * M : (k + 1) * M],
                )
                hrT.append(hrT_k)

            # --- per n-chunk: transpose tails, matmul, sigmoid, out ---
            tailsT = []
            for k in range(NK):
                tailsT_k = tailsT_pool.tile([P, B], BF16, tag=f"tailsT{k}")
                tailsT.append(tailsT_k)

            # chunk widths in tail-gather columns; narrow trailing chunks keep
            # the post-last-gather critical chain short
            widths = [4, 4, 4, 2, 1, 1]
            j0 = 0
            for w in widths:
                ncols = w * P
                for k in range(NK):
                    pst = psum_t.tile([P, ncols], BF16, tag="pst", name=f"pst_{j0}_{k}")
                    for jj in range(w):
                        j = j0 + jj
                        nc.tensor.transpose(
                            pst[:, jj * P : (jj + 1) * P],
                            tails[j][:, k * P : (k + 1) * P],
                            ident[:],
                        )
                    nc.vector.tensor_copy(
                        tailsT[k][:, j0 * P : j0 * P + ncols], pst[:]
                    )

                for i in range(NM):
                    psmm = psum_mm.tile(
                        [P, ncols], F32, tag="psmm", name=f"psmm_{j0}_{i}"
                    )
                    for k in range(NK):
                        nc.tensor.matmul(
                            psmm[:],
                            lhsT=hrT[k][:, i * P : (i + 1) * P],
                            rhs=tailsT[k][:, j0 * P : j0 * P + ncols],
                            start=(k == 0),
                            stop=(k == NK - 1),
                        )
                    o_tile = out_pool.tile(
                        [P, ncols], BF16, tag="out", name=f"out_{j0}_{i}"
                    )
                    nc.scalar.activation(
                        o_tile[:], psmm[:], mybir.ActivationFunctionType.Sigmoid
                    )
                    # alternate HWDGE engines so back-to-back output issues
                    # at the kernel tail don't serialize on one engine
                    out_eng = nc.sync if i == 0 else nc.scalar
                    out_eng.dma_start(
                        score[i * P : (i + 1) * P, j0 * P : j0 * P + ncols], o_tile[:]
                    )
                j0 += w

    nc.compile()
    return nc


_NC = None


def _get_nc():
    global _NC
    if _NC is None:
        _NC = build_nc()
    return _NC


def make_in_maps(batch_h, batch_t, batch_r, ent_emb, rel_emb):
    h = np.ascontiguousarray(np.asarray(batch_h), dtype=np.int32)
    t = np.ascontiguousarray(np.asarray(batch_t), dtype=np.int32)
    r = np.ascontiguousarray(np.asarray(batch_r), dtype=np.int32)
    ent = np.asarray(ent_emb, dtype=np.float32).astype(ml_dtypes.bfloat16)
    # rel_emb is diag_embed(rel_vec): only the diagonal carries information.
    rel_np = np.asarray(rel_emb)
    rel_diag = rel_np[:, np.arange(E), np.arange(E)].astype(np.float32)
    rel_pad = np.zeros((R_PAD, E), dtype=np.float32)
    rel_pad[:N_REL] = rel_diag
    # rel_tab[p, j*E:(j+1)*E] = rel_pad[j*128 + p]
    rel_tab = np.ascontiguousarray(
        rel_pad.reshape(NR, P, E).transpose(1, 0, 2).reshape(P, NR * E)
    ).astype(ml_dtypes.bfloat16)
    identity = np.eye(P, dtype=np.float32).astype(ml_dtypes.bfloat16)

    # partition-major wrap: idx_tile[p, g] = idx[g*128 + p]
    t_wrapped = t.reshape(NT, P).T  # (128, 16)
    in_maps = []
    for c in range(CORES):
        sl = slice(c * M, (c + 1) * M)
        idx_all = np.concatenate([h[sl].reshape(NM, P).T, t_wrapped], axis=1)
        # onehot[p, j*M + m] = 1 iff r[sl][m] == j*128 + p
        r_core = r[sl]
        oh = np.zeros((P, NR * M), dtype=ml_dtypes.bfloat16)
        jj, pp = np.divmod(r_core, P)
        oh[pp, jj * M + np.arange(M)] = 1.0
        in_maps.append(
            {
                "idx": np.ascontiguousarray(idx_all),
                "identity": identity,
                "ent_emb": ent,
                "rel_tab": rel_tab,
                "onehot": np.ascontiguousarray(oh),
            }
        )
    return in_maps


def run(batch_h, batch_t, batch_r, ent_emb, rel_emb, trace=False, tmpdir=None):
    """Run the SPMD kernel; returns (score, BassKernelResults)."""
    from concourse.bass_utils import run_bass_kernel_spmd

    nc = _get_nc()
    in_maps = make_in_maps(batch_h, batch_t, batch_r, ent_emb, rel_emb)
    kwargs = {}
    if trace:
        kwargs = {"trace": True, "tmpdir": tmpdir}
    res = run_bass_kernel_spmd(nc, in_maps, core_ids=list(range(CORES)), **kwargs)
    score = np.concatenate(
        [np.asarray(res.results[c]["score"]) for c in range(CORES)], axis=0
    ).astype(np.float32)
    return score, res


def kernel(batch_h, batch_t, batch_r, ent_emb, rel_emb):
    score, _ = run(batch_h, batch_t, batch_r, ent_emb, rel_emb)
    return score


# revision 10
# speedup vs baseline: 2.2994x; 1.0653x over previous
"""DistMult scoring kernel for Trainium2 (8 NeuronCores, SPMD 4x2 grid).

Problem: score = sigmoid( (ent_emb[h] * diag(rel_emb[r])) @ ent_emb[t].T )

v7: the (B,B) score matrix is tiled 4x2 across the 8 cores: core c=(gi,gj)
computes rows [gi*512,+512) x cols [gj*1024,+1024). Per core that needs 4
head-tile gathers + 8 tail-tile gathers = 12 SWDGE indirect DMAs (~1.4us
fixed cost each) instead of the row-split's 18 - SWDGE instruction count is
the dominant serial cost on the gpsimd engine.

Datapath is all-bf16 (fp32 PSUM accumulate): host converts ent_emb to bf16;
rel selection via one-hot matmul on the PE against a resident rel-diagonal
table; PE transposes and score matmuls in bf16; sigmoid reads fp32 PSUM and
writes bf16; both row-tiles of each score chunk leave in one 3D DMA; host
widens to f32 and assembles the 4x2 grid.
"""

import sys

if "/opt/trn_rl_repo" not in sys.path:
    sys.path.insert(0, "/opt/trn_rl_repo")

import numpy as np
import ml_dtypes

import concourse.bass as bass
import concourse.tile as tile
from concourse import bacc, mybir

B = 2048          # batch size
E = 256           # embedding dim
N_ENT = 400000
N_REL = 500
R_PAD = 512
CORES = 8
GR, GC = 4, 2     # core grid
M = B // GR       # head rows per core = 512
N = B // GC       # tail cols per core = 1024
P = 128

F32 = mybir.dt.float32
BF16 = mybir.dt.bfloat16
I32 = mybir.dt.int32

NMI = M // P      # 4 head tiles
NT = N // P       # 8 tail index columns
NK = E // P       # 2 contraction tiles
NR = R_PAD // P   # 4 rel-table k tiles

WARMUP = 20       # dummy PE matmuls to keep the HAM clock ramping


def build_nc():
    nc = bacc.Bacc("TRN2", target_bir_lowering=False, debug=False, num_devices=CORES)

    # idx layout: cols [0:4] heads, [4:12] tails
    idx = nc.dram_tensor("idx", [P, NMI + NT], I32, kind="ExternalInput").ap()
    identity = nc.dram_tensor("identity", [P, P], BF16, kind="ExternalInput").ap()
    ent = nc.dram_tensor("ent_emb", [N_ENT, E], BF16, kind="ExternalInput").ap()
    rel_tab = nc.dram_tensor("rel_tab", [P, NR * E], BF16, kind="ExternalInput").ap()
    # onehot[p, j*M + m] = 1.0 iff batch_r[row_slice][m] == j*128 + p
    onehot = nc.dram_tensor("onehot", [P, NR * M], BF16, kind="ExternalInput").ap()
    score = nc.dram_tensor("score", [M, N], BF16, kind="ExternalOutput").ap()

    with tile.TileContext(nc) as tc:
        with (
            tc.tile_pool(name="const", bufs=1) as const_pool,
            tc.tile_pool(name="idxp", bufs=1) as idx_pool,
            tc.tile_pool(name="gather", bufs=1) as gather_pool,
            tc.tile_pool(name="tailsT", bufs=1) as tailsT_pool,
            tc.tile_pool(name="outp", bufs=4) as out_pool,
            tc.tile_pool(name="pst", bufs=2, space="PSUM") as psum_t,
            tc.tile_pool(name="psmm", bufs=3, space="PSUM") as psum_mm,
            tc.tile_pool(name="psrt", bufs=1, space="PSUM") as psum_rt,
        ):
            idx_sb = idx_pool.tile([P, NMI + NT], I32)
            nc.sync.dma_start(idx_sb[:], idx[:])
            ident = const_pool.tile([P, P], BF16)
            nc.scalar.dma_start(ident[:], identity[:])
            rel_sb = const_pool.tile([P, NR * E], BF16)
            nc.scalar.dma_start(rel_sb[:], rel_tab[:])
            oh_sb = const_pool.tile([P, NR * M], BF16)
            nc.scalar.dma_start(oh_sb[:], onehot[:])

            # --- gathers: heads first (hr path is longest), then tails ---
            h_tiles = []
            for i in range(NMI):
                h_tile = gather_pool.tile([P, E], BF16, tag=f"heads{i}")
                nc.gpsimd.indirect_dma_start(
                    out=h_tile[:],
                    out_offset=None,
                    in_=ent[:],
                    in_offset=bass.IndirectOffsetOnAxis(ap=idx_sb[:, i : i + 1], axis=0),
                )
                h_tiles.append(h_tile)

            tails = [None] * NT
            for j in range(NT):
                t_tile = gather_pool.tile([P, E], BF16, tag=f"tails{j}", name=f"t{j}")
                nc.gpsimd.indirect_dma_start(
                    out=t_tile[:],
                    out_offset=None,
                    in_=ent[:],
                    in_offset=bass.IndirectOffsetOnAxis(
                        ap=idx_sb[:, NMI + j : NMI + j + 1], axis=0
                    ),
                )
                tails[j] = t_tile

            # --- PE warmup (shares the psmm bank pool) ---
            wm = psum_mm.tile([P, P], F32, tag="psmm", name="wm")
            for _ in range(WARMUP):
                nc.tensor.matmul(wm[:], lhsT=ident[:], rhs=ident[:], start=True, stop=True)

            # --- rT[k][e, m] via one-hot matmul over the rel table ---
            rt_all = psum_rt.tile([P, NK * M], F32, tag="rt", name="rt_all")
            for k in range(NK):
                for r in range(NR):
                    nc.tensor.matmul(
                        rt_all[:, k * M : (k + 1) * M],
                        lhsT=rel_sb[:, r * E + k * P : r * E + (k + 1) * P],
                        rhs=oh_sb[:, r * M : (r + 1) * M],
                        start=(r == 0),
                        stop=(r == NR - 1),
                    )

            # --- hT[k][e, m] via PE transposes of gathered head rows ---
            ht_all = psum_rt.tile([P, NK * M], BF16, tag="ht", name="ht_all")
            for k in range(NK):
                for i in range(NMI):
                    nc.tensor.transpose(
                        ht_all[:, k * M + i * P : k * M + (i + 1) * P],
                        h_tiles[i][:, k * P : (k + 1) * P],
                        ident[:],
                    )

            # --- hrT[k] = hT[k] * rT[k]; bounce hT via ScalarE (DVE reads
            # at most one PSUM operand) ---
            ht_sb = gather_pool.tile([P, NK * M], BF16, tag="ht_sb")
            nc.scalar.activation(
                ht_sb[:], ht_all[:], mybir.ActivationFunctionType.Copy
            )
            hrT = []
            for k in range(NK):
                hrT_k = gather_pool.tile([P, M], BF16, tag=f"hrT{k}")
                nc.vector.tensor_mul(
                    hrT_k[:],
                    ht_sb[:, k * M : (k + 1) * M],
                    rt_all[:, k * M : (k + 1) * M],
                )
                hrT.append(hrT_k)

            # --- per n-chunk: transpose tails, matmul, sigmoid, merged out ---
            tailsT = []
            for k in range(NK):
                tailsT_k = tailsT_pool.tile([P, N], BF16, tag=f"tailsT{k}")
                tailsT.append(tailsT_k)

            widths = [4, 2, 1, 1]
            j0 = 0
            for ci, w in enumerate(widths):
                ncols = w * P
                for k in range(NK):
                    pst = psum_t.tile([P, ncols], BF16, tag="pst", name=f"pst_{j0}_{k}")
                    for jj in range(w):
                        j = j0 + jj
                        nc.tensor.transpose(
                            pst[:, jj * P : (jj + 1) * P],
                            tails[j][:, k * P : (k + 1) * P],
                            ident[:],
                        )
                    nc.vector.tensor_copy(
                        tailsT[k][:, j0 * P : j0 * P + ncols], pst[:]
                    )

                o_tile = out_pool.tile(
                    [P, NMI, ncols], BF16, tag="out", name=f"out_{j0}"
                )
                for i in range(NMI):
                    psmm = psum_mm.tile(
                        [P, ncols], F32, tag="psmm", name=f"psmm_{j0}_{i}"
                    )
                    for k in range(NK):
                        nc.tensor.matmul(
                            psmm[:],
                            lhsT=hrT[k][:, i * P : (i + 1) * P],
                            rhs=tailsT[k][:, j0 * P : j0 * P + ncols],
                            start=(k == 0),
                            stop=(k == NK - 1),
                        )
                    nc.scalar.activation(
                        o_tile[:, i, :], psmm[:],
                        mybir.ActivationFunctionType.Sigmoid,
                    )
                out_eng = nc.sync if ci % 2 == 0 else nc.scalar
                out_eng.dma_start(
                    score[:, j0 * P : j0 * P + ncols].rearrange(
                        "(i p) c -> p i c", i=NMI
                    ),
                    o_tile[:],
                )
                j0 += w

    nc.compile()
    return nc


_NC = None


def _get_nc():
    global _NC
    if _NC is None:
        _NC = build_nc()
    return _NC


def make_in_maps(batch_h, batch_t, batch_r, ent_emb, rel_emb):
    h = np.ascontiguousarray(np.asarray(batch_h), dtype=np.int32)
    t = np.ascontiguousarray(np.asarray(batch_t), dtype=np.int32)
    r = np.ascontiguousarray(np.asarray(batch_r), dtype=np.int32)
    ent = np.asarray(ent_emb, dtype=np.float32).astype(ml_dtypes.bfloat16)
    rel_np = np.asarray(rel_emb)
    rel_diag = rel_np[:, np.arange(E), np.arange(E)].astype(np.float32)
    rel_pad = np.zeros((R_PAD, E), dtype=np.float32)
    rel_pad[:N_REL] = rel_diag
    rel_tab = np.ascontiguousarray(
        rel_pad.reshape(NR, P, E).transpose(1, 0, 2).reshape(P, NR * E)
    ).astype(ml_dtypes.bfloat16)
    identity = np.eye(P, dtype=np.float32).astype(ml_dtypes.bfloat16)

    in_maps = []
    for c in range(CORES):
        gi, gj = divmod(c, GC)
        rs = slice(gi * M, (gi + 1) * M)
        cs = slice(gj * N, (gj + 1) * N)
        idx_all = np.concatenate(
            [h[rs].reshape(NMI, P).T, t[cs].reshape(NT, P).T], axis=1
        )
        r_core = r[rs]
        oh = np.zeros((P, NR * M), dtype=ml_dtypes.bfloat16)
        jj, pp = np.divmod(r_core, P)
        oh[pp, jj * M + np.arange(M)] = 1.0
        in_maps.append(
            {
                "idx": np.ascontiguousarray(idx_all),
                "identity": identity,
                "ent_emb": ent,
                "rel_tab": rel_tab,
                "onehot": np.ascontiguousarray(oh),
            }
        )
    return in_maps


def run(batch_h, batch_t, batch_r, ent_emb, rel_emb, trace=False, tmpdir=None):
    """Run the SPMD kernel; returns (score, BassKernelResults)."""
    from concourse.bass_utils import run_bass_kernel_spmd

    nc = _get_nc()
    in_maps = make_in_maps(batch_h, batch_t, batch_r, ent_emb, rel_emb)
    kwargs = {}
    if trace:
        kwargs = {"trace": True, "tmpdir": tmpdir}
    res = run_bass_kernel_spmd(nc, in_maps, core_ids=list(range(CORES)), **kwargs)
    blocks = [np.asarray(res.results[c]["score"]) for c in range(CORES)]
    rows = [
        np.concatenate(blocks[gi * GC : (gi + 1) * GC], axis=1) for gi in range(GR)
    ]
    score = np.concatenate(rows, axis=0).astype(np.float32)
    return score, res


def kernel(batch_h, batch_t, batch_r, ent_emb, rel_emb):
    score, _ = run(batch_h, batch_t, batch_r, ent_emb, rel_emb)
    return score
